# revision 31
# baseline (speedup 1.0000x reference)
"""Trainium2 Bass kernel: TextCNN (conv k=3/4/5 over [B,1,S,E] + relu +
global max-pool + FC + log_softmax), data-parallel over batch on 8 cores.

Conv = fp32r (tf32) matmuls contracting over E, tap shifts folded into
PSUM accumulation by slicing the moving operand. The 44-row tail of the
E=300 contraction is packed two-taps-per-matmul along K using a host-
prepared shift-replicated copy of x. Host packs weights, x, pair region
and FC aux into one DRAM array per core; x streams in per-group DMAs
that overlap compute. Every instruction is kept to <=1 semaphore wait
(this toolchain's TPB encodings have a single wait slot) via dummy-matmul
fences, and the kernel-tail drain is split per semaphore proc.

Self-contained: hardcodes shapes/sharding; only imports the container
toolchain at /opt/trn_rl_repo.
"""

import sys

import numpy as np

sys.path.insert(0, "/opt/trn_rl_repo")

import concourse.bass as bass  # noqa: E402
import concourse.tile as tile  # noqa: E402
from concourse import mybir  # noqa: E402
from concourse.bass_utils import run_bass_kernel_spmd  # noqa: E402
from concourse.tile import add_dep_helper  # noqa: E402
from concourse.vector_clock import ScopedClock, VectorClock  # noqa: E402

B, S, E = 512, 128, 300
NF = 100
NCLS = 5
NCORES = 8
BPC = B // NCORES  # 64 batch elems per core
G = 4  # batch elems per matmul group (4*128 = 512 moving cols)
NG = BPC // G  # 16 groups
PAD = 2
SP = S + 2 * PAD  # 132 padded seq length
KS = (3, 4, 5)
SOUT = {3: S - 2, 4: S - 1, 5: S}  # valid conv output positions per branch
SMM = {3: S - 2, 4: S, 5: S}  # matmul cols (fp32r needs even innermost count)
TAPBASE = {3: 0, 4: 3, 5: 7}
EC01 = ((0, 128), (128, 128))  # full-K contraction chunks
E2, E2N = 256, 44  # tail chunk rows
PAIRS = ((3, 0), (4, 0), (4, 2), (5, 0), (5, 2))  # (k, first tap) packed pairs
SINGLES = {3: (2,), 4: (), 5: (4,)}  # leftover c2 taps
WCOLS = 12 * NF  # 1200 tap-major weight columns
XBASE = WCOLS  # x region starts after weights
AUXBASE = XBASE + BPC * SP  # aux (bias | wfct+bfc rows) after x
AUXW = 3 + 3 * NCLS
TOTW = AUXBASE + AUXW
PROW = 300  # pair region rows start (wpair cols 0:500, xpair in x region)
ROWS = PROW + 2 * E2N  # 388 DRAM rows

_f32 = mybir.dt.float32
_f32r = mybir.dt.float32r

_built = None


def _ins(i):
    return i.ins if hasattr(i, "ins") else i


def _dep(from_inst, to_inst, reason, sync=True):
    add_dep_helper(_ins(from_inst), _ins(to_inst), sync=sync, reason=reason)


class _SplitDrainTC(tile.TileContext):
    """TileContext whose kernel-tail drain is split into one drain per
    semaphore proc: the stock single drain carries one wait per used proc,
    which overflows the CTRL_NO encoding's wait slots on this toolchain."""

    def _drain_and_barrier(self, tick_clock, wait_clock):
        gc = tick_clock.global_clock
        ticks = eval(str(gc).replace("VectorClock", ""))
        for idx, tick in enumerate(ticks):
            if tick > 0:
                sub = VectorClock()
                sub.require_at_least(idx, tick)
                d = self.nc.sync.drain()
                wait_clock.add_sem_waits(d.ins, ScopedClock({None: sub}))
        self.nc.all_engine_barrier()
        assert self.sems is not None
        popped = self.nc._tile_sem_poison_stack.pop()
        assert popped is self._sem_poison
        self.nc.clear_and_free_semaphores(list(self.sems.allocated().values()))
        self.nc.all_engine_barrier()


def _build():
    nc = bass.Bass()
    xw = nc.declare_dram_parameter("xw", [ROWS, TOTW], _f32r, isOutput=False)
    out = nc.declare_dram_parameter("out", [NCLS, BPC], _f32, isOutput=True)

    act = mybir.ActivationFunctionType

    with _SplitDrainTC(nc) as tc:
        with (
            tc.tile_pool(name="consts", bufs=1) as consts,
            tc.tile_pool(name="xin", bufs=16) as xin,
            tc.tile_pool(name="small", bufs=4) as small,
            tc.tile_pool(name="feat", bufs=1) as featp,
            tc.tile_pool(name="psum", bufs=2, space="PSUM") as psum,
            tc.tile_pool(name="psfc", bufs=1, space="PSUM") as psfc,
        ):
            pescr = psfc.tile([128, 512], _f32, tag="pescr")
            dscr = small.tile([1, 2], _f32, tag="dscr")
            nc.vector.memset(dscr[:], 0.5)
            wt = [None, None, None]
            wp = None

            def _pe_tick():
                return nc.tensor.matmul(
                    pescr[0:1, 0:1],
                    dscr[0:1, 0:1],
                    dscr[0:1, 1:2],
                    start=True,
                    stop=True,
                )


            xtiles = {}

            def make_x(g):
                if g in xtiles:
                    return xtiles[g]
                ts, ds = [], []
                for c, (c0, pc) in enumerate(EC01 + ((PROW, 2 * E2N),)):
                    t = xin.tile([pc, G, SP], _f32r, tag=f"x{c}", name=f"x{c}_{g}")
                    ds.append(
                        nc.sync.dma_start(
                            out=t[:],
                            in_=xw[
                                c0 : c0 + pc,
                                XBASE + g * G * SP : XBASE + (g + 1) * G * SP,
                            ].rearrange("p (b s) -> p b s", b=G),
                        )
                    )
                    ts.append(t)
                xtiles[g] = (ts, ds)
                return xtiles[g]

            # prewarm: full-array fp32r dummy matmuls bridge the DMA ramp
            # so the HAM clock gate is at 8/8 when the real matmuls start
            junkf = small.tile([128, 256], _f32, tag="junkf")
            nc.vector.memset(junkf[:], 0.25)
            junk = small.tile([128, 256], _f32r, tag="junk")
            nc.vector.tensor_copy(junk[:], junkf[:])
            for _ in range(48):
                nc.tensor.matmul(
                    pescr[:, 0:256],
                    junk[:, :128],
                    junk[:, :],
                    start=True,
                    stop=True,
                )

            wdmas = []
            wdmasb = []
            for c, (c0, pc) in enumerate(EC01 + ((E2, E2N),)):
                t = consts.tile([pc, WCOLS], _f32r, tag=f"w{c}", name=f"w{c}")
                wdmas.append(
                    nc.sync.dma_start(
                        out=t[:, : 3 * NF], in_=xw[c0 : c0 + pc, : 3 * NF]
                    )
                )
                wdmasb.append(
                    nc.sync.dma_start(
                        out=t[:, 3 * NF :], in_=xw[c0 : c0 + pc, 3 * NF : WCOLS]
                    )
                )
                wt[c] = t
            wp = consts.tile([2 * E2N, 5 * NF], _f32r, tag="wp", name="wp")
            wdmas.append(
                nc.sync.dma_start(
                    out=wp[:], in_=xw[PROW : PROW + 2 * E2N, : 5 * NF]
                )
            )
            auxt = consts.tile([NF + 1, AUXW], _f32r, tag="aux", name="aux")
            aux_dma = nc.sync.dma_start(
                out=auxt[:], in_=xw[: NF + 1, AUXBASE:TOTW]
            )
            make_x(0)
            make_x(1)

            ascratch = small.tile([1, 1], _f32, tag="ascratch")

            feats = [
                featp.tile([NF, BPC], _f32, tag=f"feat{kk}", name=f"feat{kk}")
                for kk in range(3)
            ]
            featr = [
                featp.tile(
                    [NF + (1 if kk == 2 else 0), BPC],
                    _f32,
                    tag=f"featr{kk}",
                    name=f"featr{kk}",
                )
                for kk in range(3)
            ]
            nc.vector.memset(featr[2][:], 1.0)

            plT = psfc.tile([NCLS, BPC], _f32, tag="plT")
            ones5 = small.tile([NCLS, 1], _f32, tag="ones5")
            nc.vector.memset(ones5[:], 1.0)
            mones1 = small.tile([1, NCLS], _f32, tag="mones1")
            nc.vector.memset(mones1[:], -1.0)
            afence = nc.scalar.memzero(ascratch[:])
            _dep(afence, aux_dma, "act waits aux")
            # touch Exp/Ln tables now so the tail doesn't pay cold loads
            nc.scalar.activation(ascratch[:], ascratch[:], act.Exp)
            nc.scalar.activation(ascratch[:], ascratch[:], act.Ln)

            reds = {}
            last_mms = {}
            for g in range(NG):
                xtf, xdmas = make_x(g)
                h = 0

                # fence chain: split the group-start matmul's deps across
                # dummy 1x1 matmuls so real matmuls carry <=1 wait
                fence = None

                def _chain(nop, fence):
                    if fence is not None:
                        _dep(nop, fence, "chain", sync=False)
                    return nop

                if g == 0:
                    nop = _pe_tick()
                    _dep(nop, wdmas[0], "w0 loaded")
                    fence = _chain(nop, fence)
                    nop = _pe_tick()
                    _dep(nop, xdmas[0], "x0 loaded")
                    fence = _chain(nop, fence)
                else:
                    for xd in xdmas:
                        nop = _pe_tick()
                        _dep(nop, xd, "x loaded")
                        fence = _chain(nop, fence)
                if g >= 2:
                    nop = _pe_tick()
                    for r in reds[g - 2]:
                        _dep(nop, r, "psum released")
                    fence = _chain(nop, fence)
                    nop = _pe_tick()
                    for m in last_mms[g - 2]:
                        _dep(nop, m, "psum group done")
                    fence = _chain(nop, fence)

                reds[g] = []
                last_mms[g] = []
                for kk, k in enumerate(KS):
                    smm = SMM[k]
                    ps = psum.tile([NF, G, S], _f32, tag=f"y{k}", name=f"y{k}_{g}")
                    nmm = 2 * k + len([p for p in PAIRS if p[0] == k]) + len(
                        SINGLES[k]
                    )
                    n = 0

                    pend = [fence]

                    def mm_step(lhsT, rhs):
                        nonlocal n
                        m = nc.tensor.matmul(
                            ps[:, :, :smm],
                            lhsT,
                            rhs,
                            start=(n == 0),
                            stop=(n == nmm - 1),
                        )
                        if pend[0] is not None:
                            _dep(m, pend[0], "fence", sync=False)
                            pend[0] = None
                        n += 1
                        return m

                    for c in range(2):
                        if g == 0 and kk == 0 and c == 1:
                            nop = _pe_tick()
                            _dep(nop, wdmas[1], "w1 loaded")
                            nop2 = _pe_tick()
                            _dep(nop2, xdmas[1], "x1 loaded")
                            _dep(nop2, nop, "chain", sync=False)
                            pend[0] = nop2
                        for i in range(k):
                            col = (TAPBASE[k] + i) * NF
                            off = 5 - k + i
                            mm = mm_step(
                                wt[c][:, col : col + NF],
                                xtf[c][:, h : h + G, off : off + smm],
                            )
                    if g == 0 and kk == 0:
                        nop = _pe_tick()
                        _dep(nop, wdmas[3], "wp loaded")
                        nop2 = _pe_tick()
                        _dep(nop2, wdmas[2], "w2 loaded")
                        _dep(nop2, nop, "chain", sync=False)
                        nop3 = _pe_tick()
                        _dep(nop3, xdmas[2], "xp loaded")
                        _dep(nop3, nop2, "chain", sync=False)
                        pend[0] = nop3
                    for p, (pk, ta) in enumerate(PAIRS):
                        if pk != k:
                            continue
                        off = 5 - k + ta
                        mm = mm_step(
                            wp[:, p * NF : (p + 1) * NF],
                            xtf[2][:, h : h + G, off : off + smm],
                        )
                    for i in SINGLES[k]:
                        col = (TAPBASE[k] + i) * NF
                        off = 5 - k + i
                        mm = mm_step(
                            wt[2][:, col : col + NF],
                            xtf[2][:E2N, h : h + G, off : off + smm],
                        )
                    last_mms[g].append(mm)
                    red = nc.vector.reduce_max(
                        feats[kk][:, g * G : (g + 1) * G],
                        ps[:, :, : SOUT[k]],
                        axis=mybir.AxisListType.X,
                    )
                    reds[g].append(red)
                    r = nc.scalar.activation(
                        featr[kk][:NF, g * G : (g + 1) * G],
                        feats[kk][:, g * G : (g + 1) * G],
                        act.Relu,
                        bias=auxt[:NF, kk : kk + 1].bitcast(_f32),
                    )
                    _dep(r, afence, "act fence", sync=False)

                if g % 4 == 3:
                    s = g // 4
                    if s == 0:
                        nop = _pe_tick()
                        _dep(nop, aux_dma, "aux loaded for FC")
                    for kk in range(3):
                        krows = NF + (1 if kk == 2 else 0)
                        wsl = auxt[
                            :krows, 3 + NCLS * kk : 3 + NCLS * (kk + 1)
                        ].bitcast(_f32)
                        nc.tensor.matmul(
                            plT[:, 16 * s : 16 * (s + 1)],
                            wsl,
                            featr[kk][:krows, 16 * s : 16 * (s + 1)],
                            start=(s == 0 and kk == 0),
                            stop=False,
                        )

            # log_softmax in transposed layout: x - ln(sum exp x), the
            # class-dim reduction and broadcast both done with tiny matmuls
            expT = small.tile([NCLS, BPC], _f32, tag="expT")
            nc.scalar.activation(expT[:], plT[:], act.Exp)
            nc.tensor.matmul(
                pescr[0:1, 64:128], ones5[:], expT[:], start=True, stop=True
            )
            lns = small.tile([1, BPC], _f32, tag="lns")
            nc.scalar.activation(lns[:], pescr[0:1, 64:128], act.Ln)
            nc.tensor.matmul(plT[:], mones1[:], lns[:], start=False, stop=True)
            ot = small.tile([NCLS, BPC], _f32, tag="ot")
            nc.vector.tensor_copy(ot[:], plT[:])
            nc.gpsimd.dma_start(out=out[:, :], in_=ot[:])
    return nc


def _round_tf32(a):
    u = a.view(np.uint32)
    r = ((u >> 13) & 1) + 0x0FFF
    return ((u + r) & 0xFFFFE000).view(np.float32)


def _prep(x, w3, b3, w4, b4, w5, b5, Wfc, bfc):
    x = np.asarray(x, dtype=np.float32).reshape(B, S, E)
    ws = {3: np.asarray(w3, np.float32), 4: np.asarray(w4, np.float32),
          5: np.asarray(w5, np.float32)}
    base = np.zeros((ROWS, TOTW), np.float32)
    col = 0
    for k in KS:
        for i in range(k):
            base[:E, col : col + NF] = _round_tf32(
                np.ascontiguousarray(ws[k][:, 0, i, :].T)
            )
            col += NF
    # pair weights: rows PROW..PROW+44 = tap ta, +44 = tap ta+1 (E tail rows)
    for p, (k, ta) in enumerate(PAIRS):
        base[PROW : PROW + E2N, p * NF : (p + 1) * NF] = _round_tf32(
            np.ascontiguousarray(ws[k][:, 0, ta, E2:].T)
        )
        base[PROW + E2N : PROW + 2 * E2N, p * NF : (p + 1) * NF] = _round_tf32(
            np.ascontiguousarray(ws[k][:, 0, ta + 1, E2:].T)
        )
    # aux: bias cols then wfct chunks; bfc in row NF of chunk 2
    for kk, bb in enumerate((b3, b4, b5)):
        base[:NF, AUXBASE + kk] = np.asarray(bb, np.float32)
    Wfc = np.asarray(Wfc, np.float32)
    for kk in range(3):
        base[:NF, AUXBASE + 3 + NCLS * kk : AUXBASE + 3 + NCLS * (kk + 1)] = Wfc[
            :, kk * NF : (kk + 1) * NF
        ].T
    base[NF, AUXBASE + 3 + 2 * NCLS : AUXBASE + 3 + 3 * NCLS] = np.asarray(
        bfc, np.float32
    )

    xt_all = np.zeros((E, B, SP), np.float32)
    xt_all[:, :, PAD : PAD + S] = _round_tf32(
        np.ascontiguousarray(x.transpose(2, 0, 1))
    )
    shards = []
    for c in range(NCORES):
        arr = base.copy()
        xc = xt_all[:, c * BPC : (c + 1) * BPC, :]
        arr[:E2, XBASE:AUXBASE] = xc[:E2].reshape(E2, -1)
        # pair x: shift-0 rows then shift-1 rows of the E tail
        tail = xc[E2:]
        arr[PROW : PROW + E2N, XBASE:AUXBASE] = tail.reshape(E2N, -1)
        sh = np.zeros_like(tail)
        sh[:, :, :-1] = tail[:, :, 1:]
        arr[PROW + E2N : PROW + 2 * E2N, XBASE:AUXBASE] = sh.reshape(E2N, -1)
        shards.append(arr)
    return shards


def _run(inputs, **spmd_kwargs):
    global _built
    if _built is None:
        _built = _build()
    shards = _prep(**inputs)
    in_maps = [{"xw": shards[c]} for c in range(NCORES)]
    res = run_bass_kernel_spmd(_built, in_maps, list(range(NCORES)), **spmd_kwargs)
    outp = np.concatenate(
        [np.asarray(res.results[c]["out"]).T for c in range(NCORES)], axis=0
    )
    return outp, res


def kernel(**inputs):
    outp, _ = _run(inputs)
    return outp


# revision 32
# speedup vs baseline: 1.0304x; 1.0304x over previous
"""Trainium2 Bass kernel: TextCNN (conv k=3/4/5 over [B,1,S,E] + relu +
global max-pool + FC + log_softmax), data-parallel over batch on 8 cores.

Conv = fp32r (tf32) matmuls contracting over E, tap shifts folded into
PSUM accumulation by slicing the moving operand. The 44-row tail of the
E=300 contraction is packed two-taps-per-matmul along K using a host-
prepared shift-replicated copy of x. Host packs weights, x, pair region
and FC aux into one DRAM array per core; x streams in per-group DMAs
that overlap compute. Every instruction is kept to <=1 semaphore wait
(this toolchain's TPB encodings have a single wait slot) via dummy-matmul
fences, and the kernel-tail drain is split per semaphore proc.

Self-contained: hardcodes shapes/sharding; only imports the container
toolchain at /opt/trn_rl_repo.
"""

import sys

import numpy as np

sys.path.insert(0, "/opt/trn_rl_repo")

import concourse.bass as bass  # noqa: E402
import concourse.tile as tile  # noqa: E402
from concourse import mybir  # noqa: E402
from concourse.bass_utils import run_bass_kernel_spmd  # noqa: E402
from concourse.tile import add_dep_helper  # noqa: E402
from concourse.vector_clock import ScopedClock, VectorClock  # noqa: E402

B, S, E = 512, 128, 300
NF = 100
NCLS = 5
NCORES = 8
BPC = B // NCORES  # 64 batch elems per core
G = 4  # batch elems per matmul group (4*128 = 512 moving cols)
NG = BPC // G  # 16 groups
PAD = 2
SP = S + 2 * PAD  # 132 padded seq length
KS = (3, 4, 5)
SOUT = {3: S - 2, 4: S - 1, 5: S}  # valid conv output positions per branch
SMM = {3: S - 2, 4: S, 5: S}  # matmul cols (fp32r needs even innermost count)
TAPBASE = {3: 0, 4: 3, 5: 7}
EC01 = ((0, 128), (128, 128))  # full-K contraction chunks
E2, E2N = 256, 44  # tail chunk rows
PAIRS = ((3, 0), (4, 0), (4, 2), (5, 0), (5, 2))  # (k, first tap) packed pairs
SINGLES = {3: (2,), 4: (), 5: (4,)}  # leftover c2 taps
WCOLS = 12 * NF  # 1200 tap-major weight columns
XBASE = WCOLS  # x region starts after weights
AUXBASE = XBASE + BPC * SP  # aux (bias | wfct+bfc rows) after x
AUXW = 3 + 3 * NCLS
TOTW = AUXBASE + AUXW
PROW = 300  # pair region rows start (wpair cols 0:500, xpair in x region)
ROWS = PROW + 2 * E2N  # 388 DRAM rows

_f32 = mybir.dt.float32
_f32r = mybir.dt.float32r

_built = None


def _ins(i):
    return i.ins if hasattr(i, "ins") else i


def _dep(from_inst, to_inst, reason, sync=True):
    add_dep_helper(_ins(from_inst), _ins(to_inst), sync=sync, reason=reason)


class _SplitDrainTC(tile.TileContext):
    """TileContext whose kernel-tail drain is split into one drain per
    semaphore proc: the stock single drain carries one wait per used proc,
    which overflows the CTRL_NO encoding's wait slots on this toolchain."""

    def _drain_and_barrier(self, tick_clock, wait_clock):
        gc = tick_clock.global_clock
        ticks = eval(str(gc).replace("VectorClock", ""))
        for idx, tick in enumerate(ticks):
            if tick > 0:
                sub = VectorClock()
                sub.require_at_least(idx, tick)
                d = self.nc.sync.drain()
                wait_clock.add_sem_waits(d.ins, ScopedClock({None: sub}))
        self.nc.all_engine_barrier()
        assert self.sems is not None
        popped = self.nc._tile_sem_poison_stack.pop()
        assert popped is self._sem_poison
        self.nc.clear_and_free_semaphores(list(self.sems.allocated().values()))
        self.nc.all_engine_barrier()


def _build():
    nc = bass.Bass()
    xw = nc.declare_dram_parameter("xw", [ROWS, TOTW], _f32r, isOutput=False)
    out = nc.declare_dram_parameter("out", [NCLS, BPC], _f32, isOutput=True)

    act = mybir.ActivationFunctionType

    with _SplitDrainTC(nc) as tc:
        with (
            tc.tile_pool(name="consts", bufs=1) as consts,
            tc.tile_pool(name="xin", bufs=16) as xin,
            tc.tile_pool(name="small", bufs=4) as small,
            tc.tile_pool(name="feat", bufs=1) as featp,
            tc.tile_pool(name="psum", bufs=2, space="PSUM") as psum,
            tc.tile_pool(name="psfc", bufs=1, space="PSUM") as psfc,
        ):
            pescr = psfc.tile([128, 512], _f32, tag="pescr")
            dscr = small.tile([1, 2], _f32, tag="dscr")
            nc.vector.memset(dscr[:], 0.5)
            wt = [None, None, None]
            wp = None

            def _pe_tick():
                return nc.tensor.matmul(
                    pescr[0:1, 0:1],
                    dscr[0:1, 0:1],
                    dscr[0:1, 1:2],
                    start=True,
                    stop=True,
                )


            xtiles = {}

            def make_x(g):
                if g in xtiles:
                    return xtiles[g]
                ts, ds = [], []
                for c, (c0, pc) in enumerate(EC01 + ((PROW, 2 * E2N),)):
                    t = xin.tile([pc, G, SP], _f32r, tag=f"x{c}", name=f"x{c}_{g}")
                    ds.append(
                        nc.sync.dma_start(
                            out=t[:],
                            in_=xw[
                                c0 : c0 + pc,
                                XBASE + g * G * SP : XBASE + (g + 1) * G * SP,
                            ].rearrange("p (b s) -> p b s", b=G),
                        )
                    )
                    ts.append(t)
                xtiles[g] = (ts, ds)
                return xtiles[g]

            # prewarm: full-array fp32r dummy matmuls bridge the DMA ramp
            # so the HAM clock gate is at 8/8 when the real matmuls start
            junkf = small.tile([128, 512], _f32, tag="junkf")
            nc.vector.memset(junkf[:], 0.25)
            junk = small.tile([128, 512], _f32r, tag="junk")
            nc.vector.tensor_copy(junk[:], junkf[:])
            for _ in range(24):
                nc.tensor.matmul(
                    pescr[:, :],
                    junk[:, :128],
                    junk[:, :],
                    start=True,
                    stop=True,
                )

            wdmas = []
            for c, (c0, pc) in enumerate(EC01 + ((E2, E2N),)):
                t = consts.tile([pc, WCOLS], _f32r, tag=f"w{c}", name=f"w{c}")
                wdmas.append(
                    nc.sync.dma_start(out=t[:], in_=xw[c0 : c0 + pc, :WCOLS])
                )
                wt[c] = t
            wp = consts.tile([2 * E2N, 5 * NF], _f32r, tag="wp", name="wp")
            wdmas.append(
                nc.sync.dma_start(
                    out=wp[:], in_=xw[PROW : PROW + 2 * E2N, : 5 * NF]
                )
            )
            auxt = consts.tile([NF + 1, AUXW], _f32r, tag="aux", name="aux")
            aux_dma = nc.sync.dma_start(
                out=auxt[:], in_=xw[: NF + 1, AUXBASE:TOTW]
            )
            make_x(0)
            make_x(1)

            ascratch = small.tile([1, 1], _f32, tag="ascratch")

            feats = [
                featp.tile([NF, BPC], _f32, tag=f"feat{kk}", name=f"feat{kk}")
                for kk in range(3)
            ]
            featr = [
                featp.tile(
                    [NF + (1 if kk == 2 else 0), BPC],
                    _f32,
                    tag=f"featr{kk}",
                    name=f"featr{kk}",
                )
                for kk in range(3)
            ]
            nc.vector.memset(featr[2][:], 1.0)

            plT = psfc.tile([NCLS, BPC], _f32, tag="plT")
            ones5 = small.tile([NCLS, 1], _f32, tag="ones5")
            nc.vector.memset(ones5[:], 1.0)
            mones1 = small.tile([1, NCLS], _f32, tag="mones1")
            nc.vector.memset(mones1[:], -1.0)
            afence = nc.scalar.memzero(ascratch[:])
            _dep(afence, aux_dma, "act waits aux")
            # touch Exp/Ln tables now so the tail doesn't pay cold loads
            nc.scalar.activation(ascratch[:], ascratch[:], act.Exp)
            nc.scalar.activation(ascratch[:], ascratch[:], act.Ln)

            reds = {}
            last_mms = {}
            for g in range(NG):
                xtf, xdmas = make_x(g)
                h = 0

                # fence chain: split the group-start matmul's deps across
                # dummy 1x1 matmuls so real matmuls carry <=1 wait
                fence = None

                def _chain(nop, fence):
                    if fence is not None:
                        _dep(nop, fence, "chain", sync=False)
                    return nop

                if g == 0:
                    nop = _pe_tick()
                    _dep(nop, wdmas[0], "w0 loaded")
                    fence = _chain(nop, fence)
                    nop = _pe_tick()
                    _dep(nop, xdmas[0], "x0 loaded")
                    fence = _chain(nop, fence)
                else:
                    for xd in xdmas:
                        nop = _pe_tick()
                        _dep(nop, xd, "x loaded")
                        fence = _chain(nop, fence)
                if g >= 2:
                    nop = _pe_tick()
                    for r in reds[g - 2]:
                        _dep(nop, r, "psum released")
                    fence = _chain(nop, fence)
                    nop = _pe_tick()
                    for m in last_mms[g - 2]:
                        _dep(nop, m, "psum group done")
                    fence = _chain(nop, fence)

                reds[g] = []
                last_mms[g] = []
                for kk, k in enumerate(KS):
                    smm = SMM[k]
                    ps = psum.tile([NF, G, S], _f32, tag=f"y{k}", name=f"y{k}_{g}")
                    nmm = 2 * k + len([p for p in PAIRS if p[0] == k]) + len(
                        SINGLES[k]
                    )
                    n = 0

                    pend = [fence]

                    def mm_step(lhsT, rhs):
                        nonlocal n
                        m = nc.tensor.matmul(
                            ps[:, :, :smm],
                            lhsT,
                            rhs,
                            start=(n == 0),
                            stop=(n == nmm - 1),
                        )
                        if pend[0] is not None:
                            _dep(m, pend[0], "fence", sync=False)
                            pend[0] = None
                        n += 1
                        return m

                    for c in range(2):
                        if g == 0 and kk == 0 and c == 1:
                            nop = _pe_tick()
                            _dep(nop, wdmas[1], "w1 loaded")
                            nop2 = _pe_tick()
                            _dep(nop2, xdmas[1], "x1 loaded")
                            _dep(nop2, nop, "chain", sync=False)
                            pend[0] = nop2
                        for i in range(k):
                            col = (TAPBASE[k] + i) * NF
                            off = 5 - k + i
                            mm = mm_step(
                                wt[c][:, col : col + NF],
                                xtf[c][:, h : h + G, off : off + smm],
                            )
                    if g == 0 and kk == 0:
                        nop = _pe_tick()
                        _dep(nop, wdmas[3], "wp loaded")
                        nop2 = _pe_tick()
                        _dep(nop2, wdmas[2], "w2 loaded")
                        _dep(nop2, nop, "chain", sync=False)
                        nop3 = _pe_tick()
                        _dep(nop3, xdmas[2], "xp loaded")
                        _dep(nop3, nop2, "chain", sync=False)
                        pend[0] = nop3
                    for p, (pk, ta) in enumerate(PAIRS):
                        if pk != k:
                            continue
                        off = 5 - k + ta
                        mm = mm_step(
                            wp[:, p * NF : (p + 1) * NF],
                            xtf[2][:, h : h + G, off : off + smm],
                        )
                    for i in SINGLES[k]:
                        col = (TAPBASE[k] + i) * NF
                        off = 5 - k + i
                        mm = mm_step(
                            wt[2][:, col : col + NF],
                            xtf[2][:E2N, h : h + G, off : off + smm],
                        )
                    last_mms[g].append(mm)
                    red = nc.vector.reduce_max(
                        feats[kk][:, g * G : (g + 1) * G],
                        ps[:, :, : SOUT[k]],
                        axis=mybir.AxisListType.X,
                    )
                    reds[g].append(red)
                    r = nc.scalar.activation(
                        featr[kk][:NF, g * G : (g + 1) * G],
                        feats[kk][:, g * G : (g + 1) * G],
                        act.Relu,
                        bias=auxt[:NF, kk : kk + 1].bitcast(_f32),
                    )
                    _dep(r, afence, "act fence", sync=False)

                if g % 4 == 3:
                    s = g // 4
                    if s == 0:
                        nop = _pe_tick()
                        _dep(nop, aux_dma, "aux loaded for FC")
                    for kk in range(3):
                        krows = NF + (1 if kk == 2 else 0)
                        wsl = auxt[
                            :krows, 3 + NCLS * kk : 3 + NCLS * (kk + 1)
                        ].bitcast(_f32)
                        nc.tensor.matmul(
                            plT[:, 16 * s : 16 * (s + 1)],
                            wsl,
                            featr[kk][:krows, 16 * s : 16 * (s + 1)],
                            start=(s == 0 and kk == 0),
                            stop=False,
                        )

            # log_softmax in transposed layout: x - ln(sum exp x), the
            # class-dim reduction and broadcast both done with tiny matmuls
            expT = small.tile([NCLS, BPC], _f32, tag="expT")
            nc.scalar.activation(expT[:], plT[:], act.Exp)
            nc.tensor.matmul(
                pescr[0:1, 64:128], ones5[:], expT[:], start=True, stop=True
            )
            lns = small.tile([1, BPC], _f32, tag="lns")
            nc.scalar.activation(lns[:], pescr[0:1, 64:128], act.Ln)
            nc.tensor.matmul(plT[:], mones1[:], lns[:], start=False, stop=True)
            ot = small.tile([NCLS, BPC], _f32, tag="ot")
            nc.vector.tensor_copy(ot[:], plT[:])
            nc.gpsimd.dma_start(out=out[:, :], in_=ot[:])
    return nc


def _round_tf32(a):
    u = a.view(np.uint32)
    r = ((u >> 13) & 1) + 0x0FFF
    return ((u + r) & 0xFFFFE000).view(np.float32)


def _prep(x, w3, b3, w4, b4, w5, b5, Wfc, bfc):
    x = np.asarray(x, dtype=np.float32).reshape(B, S, E)
    ws = {3: np.asarray(w3, np.float32), 4: np.asarray(w4, np.float32),
          5: np.asarray(w5, np.float32)}
    base = np.zeros((ROWS, TOTW), np.float32)
    col = 0
    for k in KS:
        for i in range(k):
            base[:E, col : col + NF] = _round_tf32(
                np.ascontiguousarray(ws[k][:, 0, i, :].T)
            )
            col += NF
    # pair weights: rows PROW..PROW+44 = tap ta, +44 = tap ta+1 (E tail rows)
    for p, (k, ta) in enumerate(PAIRS):
        base[PROW : PROW + E2N, p * NF : (p + 1) * NF] = _round_tf32(
            np.ascontiguousarray(ws[k][:, 0, ta, E2:].T)
        )
        base[PROW + E2N : PROW + 2 * E2N, p * NF : (p + 1) * NF] = _round_tf32(
            np.ascontiguousarray(ws[k][:, 0, ta + 1, E2:].T)
        )
    # aux: bias cols then wfct chunks; bfc in row NF of chunk 2
    for kk, bb in enumerate((b3, b4, b5)):
        base[:NF, AUXBASE + kk] = np.asarray(bb, np.float32)
    Wfc = np.asarray(Wfc, np.float32)
    for kk in range(3):
        base[:NF, AUXBASE + 3 + NCLS * kk : AUXBASE + 3 + NCLS * (kk + 1)] = Wfc[
            :, kk * NF : (kk + 1) * NF
        ].T
    base[NF, AUXBASE + 3 + 2 * NCLS : AUXBASE + 3 + 3 * NCLS] = np.asarray(
        bfc, np.float32
    )

    xt_all = np.zeros((E, B, SP), np.float32)
    xt_all[:, :, PAD : PAD + S] = _round_tf32(
        np.ascontiguousarray(x.transpose(2, 0, 1))
    )
    shards = []
    for c in range(NCORES):
        arr = base.copy()
        xc = xt_all[:, c * BPC : (c + 1) * BPC, :]
        arr[:E2, XBASE:AUXBASE] = xc[:E2].reshape(E2, -1)
        # pair x: shift-0 rows then shift-1 rows of the E tail
        tail = xc[E2:]
        arr[PROW : PROW + E2N, XBASE:AUXBASE] = tail.reshape(E2N, -1)
        sh = np.zeros_like(tail)
        sh[:, :, :-1] = tail[:, :, 1:]
        arr[PROW + E2N : PROW + 2 * E2N, XBASE:AUXBASE] = sh.reshape(E2N, -1)
        shards.append(arr)
    return shards


def _run(inputs, **spmd_kwargs):
    global _built
    if _built is None:
        _built = _build()
    shards = _prep(**inputs)
    in_maps = [{"xw": shards[c]} for c in range(NCORES)]
    res = run_bass_kernel_spmd(_built, in_maps, list(range(NCORES)), **spmd_kwargs)
    outp = np.concatenate(
        [np.asarray(res.results[c]["out"]).T for c in range(NCORES)], axis=0
    )
    return outp, res


def kernel(**inputs):
    outp, _ = _run(inputs)
    return outp
